# revision 11
# baseline (speedup 1.0000x reference)
"""Causal attention head (B=4, S=4096, D=512, E=64) on 8 TRN2 NeuronCores.

Sharding: per batch b, core pair (2b, 2b+1) with zig-zag query blocks.
 - Each core projects K/V for the FULL sequence and its own 2048 queries.
 - X^T is loaded in 512-column sequence waves through small rotating wave
   pools (bufs=3), so DMA issue is paced by consumption and early waves
   land first: projections and early attention blocks overlap the load.
 - Flash-style attention with transposed scores S^T = K_chunk @ Q^T, exp on
   ScalarE (the critical engine), PV accumulated with a ones-column appended
   to V so the softmax denominator falls out of the same matmul.
 - K^T is stored packed: even key chunks in partitions 0:64, odd chunks in
   64:128, so the two concurrently-tiled score matmuls read disjoint
   partition halves without duplicating K. Q^T is duplicated across halves.
 - All PSUM->SBUF copies run on VectorE to keep ScalarE exclusively on exp.
All matmul inputs are bf16 (inputs pre-cast on host; X^T pre-transposed on
host). Output f32.
"""

import sys

sys.path.insert(0, "/opt/trn_rl_repo")

import numpy as np
import ml_dtypes

from concourse import bacc, mybir
from concourse import tile
from concourse.bass_utils import run_bass_kernel_spmd

BF16 = ml_dtypes.bfloat16
F32 = mybir.dt.float32
BF = mybir.dt.bfloat16

B, S, D, E = 4, 4096, 512, 64
P = 128
NQ = 2048          # queries per core
QBLK = 512         # query block
NCHUNK_D = D // P  # 4 contraction chunks for projections
NKCH = S // P      # 32 key chunks in the full sequence
QSTARTS = {0: [0, 1024, 2048, 3072], 1: [512, 1536, 2560, 3584]}
SLOT_J = [8, 16, 24, 32]  # uniform per-slot key-chunk counts (all cores)
WAVE = 512         # sequence positions per DMA wave

_CACHE = {}
LAST_RESULT = None


def _build():
    nc = bacc.Bacc(
        "TRN2",
        target_bir_lowering=False,
        debug=False,
        enable_asserts=True,
        num_devices=8,
    )

    xqt_d = nc.declare_dram_parameter("xqt", [D, NQ], BF, isOutput=False)
    xkt_d = nc.declare_dram_parameter("xkt", [D, S], BF, isOutput=False)
    xvt_d = nc.declare_dram_parameter("xvt", [D, S], BF, isOutput=False)
    wq = nc.declare_dram_parameter("wq", [D, E], BF, isOutput=False)  # pre-scaled 1/8
    wk = nc.declare_dram_parameter("wk", [D, E], BF, isOutput=False)
    wv = nc.declare_dram_parameter("wv", [D, E], BF, isOutput=False)
    masks = nc.declare_dram_parameter("masks", [P, 8 * QBLK], BF, isOutput=False)
    zout = nc.declare_dram_parameter("z", [E, NQ], F32, isOutput=True)

    with tile.TileContext(nc) as tc:
        with (
            tc.tile_pool(name="const", bufs=1) as const,
            tc.tile_pool(name="xw", bufs=3) as xw,
            tc.tile_pool(name="proj", bufs=1) as proj,
            tc.tile_pool(name="work", bufs=3) as work,
            tc.tile_pool(name="epi", bufs=2) as epi,
            tc.tile_pool(name="psS", bufs=2, space="PSUM") as psS,
            tc.tile_pool(name="psZ", bufs=1, space="PSUM") as psZ,
            tc.tile_pool(name="psM", bufs=3, space="PSUM") as psM,
        ):
            # ---- weights first (small, needed by every projection) ----
            wq_sb = const.tile([P, NCHUNK_D, E], BF, tag="wq")
            wk_sb = const.tile([P, NCHUNK_D, E], BF, tag="wk")
            wv_sb = const.tile([P, NCHUNK_D, E], BF, tag="wv")
            for w_dram, w_sb in ((wk, wk_sb), (wv, wv_sb), (wq, wq_sb)):
                nc.sync.dma_start(
                    out=w_sb[:, :, :],
                    in_=w_dram.rearrange("(c p) e -> p c e", p=P),
                )
            masks_sb = const.tile([P, 8 * QBLK], BF, tag="masks")

            # ---- X^T wave tiles: DMA paced by slot rotation ----
            kwaves, vwaves, qwaves = [], [], []

            def x_wave(src_d, w, waves, tag):
                t = xw.tile([P, NCHUNK_D, WAVE], BF, tag=tag)
                nc.sync.dma_start(
                    out=t[:, :, :],
                    in_=src_d[:, w * WAVE : (w + 1) * WAVE].rearrange(
                        "(c p) r -> p c r", p=P
                    ),
                )
                waves.append(t)

            # priority-ordered issue: attn block 0 deps first (k0,k1,q0,masks),
            # then remaining waves, early seq positions first
            x_wave(xkt_d, 0, kwaves, "kx")
            x_wave(xkt_d, 1, kwaves, "kx")
            x_wave(xqt_d, 0, qwaves, "qx")
            nc.sync.dma_start(out=masks_sb[:, :], in_=masks[:, :])
            x_wave(xvt_d, 0, vwaves, "vx")
            x_wave(xvt_d, 1, vwaves, "vx")
            x_wave(xqt_d, 1, qwaves, "qx")
            for w in range(2, S // WAVE):
                x_wave(xkt_d, w, kwaves, "kx")
                x_wave(xvt_d, w, vwaves, "vx")
                if w < NQ // WAVE:
                    x_wave(xqt_d, w, qwaves, "qx")

            # ---- projections ----
            # kt2 packed: even key chunk j=2m -> rows 0:64 cols [m*128,(m+1)*128)
            #             odd  key chunk j=2m+1 -> rows 64:128 same cols
            kt2 = proj.tile([P, S // 2], BF, tag="ktp")
            qt2 = proj.tile([P, NQ], BF, tag="qt")  # both halves = Q^T
            vp = proj.tile([P, NKCH, E + 1], BF, tag="vp")
            nc.gpsimd.memset(vp[:, :, E : E + 1], 1.0)

            def proj_k_block(g):
                # keys [g*512, (g+1)*512) -> kt2 packed cols [g*256, (g+1)*256)
                ps = psM.tile([E, QBLK], F32, tag="m")
                for c in range(NCHUNK_D):
                    nc.tensor.matmul(
                        ps,
                        lhsT=wk_sb[:, c, :],
                        rhs=kwaves[g][:, c, :],
                        start=(c == 0),
                        stop=(c == NCHUNK_D - 1),
                    )
                psv = ps.rearrange("e (m h x) -> e h m x", h=2, x=P)
                for h in range(2):
                    dst = kt2[
                        h * E : (h + 1) * E,
                        g * (QBLK // 2) : (g + 1) * (QBLK // 2),
                    ].rearrange("e (m x) -> e m x", x=P)
                    nc.vector.tensor_copy(dst, psv[:, h])

            def proj_q_block(g):
                ps = psM.tile([E, QBLK], F32, tag="m")
                for c in range(NCHUNK_D):
                    nc.tensor.matmul(
                        ps,
                        lhsT=wq_sb[:, c, :],
                        rhs=qwaves[g][:, c, :],
                        start=(c == 0),
                        stop=(c == NCHUNK_D - 1),
                    )
                nc.vector.tensor_copy(qt2[0:E, g * QBLK : (g + 1) * QBLK], ps)
                nc.vector.tensor_copy(qt2[E : 2 * E, g * QBLK : (g + 1) * QBLK], ps)

            def proj_v_tile(i):
                ps = psM.tile([P, E], F32, tag="m")
                w, r = i // 4, i % 4
                for c in range(NCHUNK_D):
                    nc.tensor.matmul(
                        ps,
                        lhsT=vwaves[w][:, c, r * P : (r + 1) * P],
                        rhs=wv_sb[:, c, :],
                        start=(c == 0),
                        stop=(c == NCHUNK_D - 1),
                    )
                nc.vector.tensor_copy(vp[:, i, 0:E], ps)

            # emit all projections in wave/consumption order (higher PE
            # priority than attention, so the load pipeline keeps draining)
            for g in range(S // QBLK):
                proj_k_block(g)
                for i in range(4 * g, 4 * g + 4):
                    proj_v_tile(i)
                if g < NQ // QBLK:
                    proj_q_block(g)

            # ---- attention ----
            def attn_block(ib):
                jmax = SLOT_J[ib]
                qloc = ib * QBLK
                zps = psZ.tile([E + 1, QBLK], F32, tag="zt")

                def emit_pv(pt, jp):
                    for h in range(2):
                        j = 2 * jp + h
                        nc.tensor.matmul(
                            zps,
                            lhsT=vp[:, j, :],
                            rhs=pt[:, h * QBLK : (h + 1) * QBLK],
                            start=(j == 0),
                            stop=(j == jmax - 1),
                            skip_group_check=True,
                        )

                prev = None
                for jp in range(jmax // 2):
                    sps = psS.tile([P, 2 * QBLK], F32, tag="st")
                    for h in range(2):
                        nc.tensor.matmul(
                            sps[:, h * QBLK : (h + 1) * QBLK],
                            lhsT=kt2[h * E : (h + 1) * E, jp * P : (jp + 1) * P],
                            rhs=qt2[h * E : (h + 1) * E, qloc : qloc + QBLK],
                            start=True,
                            stop=True,
                            tile_position=(h * E, 0),
                        )
                    pt = work.tile([P, 2 * QBLK], BF, tag="pt")
                    nc.scalar.activation(
                        out=pt, in_=sps, func=mybir.ActivationFunctionType.Exp
                    )
                    j0 = 2 * jp
                    if j0 >= jmax - 8:
                        m = j0 - (jmax - 8)
                        nc.vector.tensor_mul(
                            pt, pt, masks_sb[:, m * QBLK : (m + 2) * QBLK]
                        )
                    if prev is not None:
                        emit_pv(*prev)
                    prev = (pt, jp)
                emit_pv(*prev)

                # normalize in transposed layout; host undoes the transpose
                rc = epi.tile([1, QBLK], F32, tag="rc")
                nc.vector.reciprocal(rc, zps[E : E + 1, :])
                rb = epi.tile([E, QBLK], F32, tag="rb")
                nc.gpsimd.partition_broadcast(rb, rc)
                zt = epi.tile([E, QBLK], F32, tag="zt")
                nc.vector.tensor_mul(zt, zps[0:E, :], rb)
                nc.sync.dma_start(out=zout[:, qloc : qloc + QBLK], in_=zt)

            for ib in range(4):
                attn_block(ib)

    nc.compile()
    return nc


def _get_nc():
    if "nc" not in _CACHE:
        _CACHE["nc"] = _build()
    return _CACHE["nc"]


def _ensure_ntff_hook():
    """Install antenv.axon_hooks + NTFF profile hook if the image lacks it."""
    import types

    try:
        from antenv import axon_hooks  # noqa: F401

        return
    except ImportError:
        pass
    import antenv
    from concourse import bass_utils as _bu

    mod = types.ModuleType("antenv.axon_hooks")
    _state = {}
    mod.set_axon_ntff_profile_hook = lambda h: _state.__setitem__("h", h)
    mod.get_axon_ntff_profile_hook = lambda: _state.get("h")
    sys.modules["antenv.axon_hooks"] = mod
    antenv.axon_hooks = mod
    sys.path.insert(0, "/root/.axon_site/trn_agent_boot")
    from trn_boot import _ntff_profile_via_ctypes

    mod.set_axon_ntff_profile_hook(
        _ntff_profile_via_ctypes("/opt/axon/libaxon_pjrt.so")
    )
    _bu.upload_artifacts = lambda tmpdir: f"local://{tmpdir}"


def _make_masks(h):
    kl = np.arange(P)[:, None]
    ql = np.arange(QBLK)[None, :]
    diag = [(kl <= ql - P * t).astype(np.float32) for t in range(4)]
    ones = np.ones((P, QBLK), np.float32)
    zero = np.zeros((P, QBLK), np.float32)
    tiles = diag + [zero] * 4 if h == 0 else [ones] * 4 + diag
    return np.concatenate(tiles, axis=1).astype(BF16)


def kernel(key_inputs, value_inputs, query_inputs, Wq, Wk, Wv):
    global LAST_RESULT
    import os

    key_inputs = np.asarray(key_inputs, dtype=np.float32)
    value_inputs = np.asarray(value_inputs, dtype=np.float32)
    query_inputs = np.asarray(query_inputs, dtype=np.float32)
    wq_b = (np.asarray(Wq, dtype=np.float32) * 0.125).astype(BF16)
    wk_b = np.asarray(Wk, dtype=np.float32).astype(BF16)
    wv_b = np.asarray(Wv, dtype=np.float32).astype(BF16)
    masks_np = [_make_masks(0), _make_masks(1)]

    in_maps = []
    for c in range(8):
        b, h = c // 2, c % 2
        xq_c = np.concatenate(
            [query_inputs[b, q0 : q0 + QBLK] for q0 in QSTARTS[h]], axis=0
        )
        xk_c = key_inputs[b]
        xv_c = value_inputs[b]
        in_maps.append(
            {
                "xqt": np.ascontiguousarray(xq_c.T).astype(BF16),
                "xkt": np.ascontiguousarray(xk_c.T).astype(BF16),
                "xvt": np.ascontiguousarray(xv_c.T).astype(BF16),
                "wq": wq_b,
                "wk": wk_b,
                "wv": wv_b,
                "masks": masks_np[h],
            }
        )

    nc = _get_nc()
    trace = bool(int(os.environ.get("KERNEL_TRACE", "0")))
    if trace:
        _ensure_ntff_hook()
    res = run_bass_kernel_spmd(
        nc,
        in_maps,
        core_ids=list(range(8)),
        trace=trace,
        tmpdir=os.environ.get("KERNEL_TRACE_DIR") or None,
    )
    LAST_RESULT = res

    out = np.empty((B, S, E), dtype=np.float32)
    for c in range(8):
        b, h = c // 2, c % 2
        z = np.asarray(res.results[c]["z"], dtype=np.float32)  # [E, NQ]
        for ib, q0 in enumerate(QSTARTS[h]):
            out[b, q0 : q0 + QBLK] = z[:, ib * QBLK : (ib + 1) * QBLK].T
    return out


# revision 13
# speedup vs baseline: 1.0872x; 1.0872x over previous
"""Causal attention head (B=4, S=4096, D=512, E=64) on 8 TRN2 NeuronCores.

Sharding: per batch b, core pair (2b, 2b+1) with zig-zag query blocks.
 - Each core projects K/V for the FULL sequence and its own 2048 queries.
 - X^T is loaded in 512-column sequence waves through small rotating wave
   pools (bufs=3), so DMA issue is paced by consumption and early waves
   land first: projections and early attention blocks overlap the load.
 - Flash-style attention with transposed scores S^T = K_chunk @ Q^T, exp on
   ScalarE (the critical engine), PV accumulated with a ones-column appended
   to V so the softmax denominator falls out of the same matmul.
 - K^T is stored packed: even key chunks in partitions 0:64, odd chunks in
   64:128, so the two concurrently-tiled score matmuls read disjoint
   partition halves without duplicating K. Q^T is duplicated across halves.
 - All PSUM->SBUF copies run on VectorE to keep ScalarE exclusively on exp.
All matmul inputs are bf16 (inputs pre-cast on host; X^T pre-transposed on
host). Output f32.
"""

import sys

sys.path.insert(0, "/opt/trn_rl_repo")

import numpy as np
import ml_dtypes

from concourse import bacc, mybir
from concourse import tile
from concourse.bass_utils import run_bass_kernel_spmd

BF16 = ml_dtypes.bfloat16
F32 = mybir.dt.float32
BF = mybir.dt.bfloat16

B, S, D, E = 4, 4096, 512, 64
P = 128
NQ = 2048          # queries per core
QBLK = 512         # query block
NCHUNK_D = D // P  # 4 contraction chunks for projections
NKCH = S // P      # 32 key chunks in the full sequence
QSTARTS = {0: [0, 1024, 2048, 3072], 1: [512, 1536, 2560, 3584]}
SLOT_J = [8, 16, 24, 32]  # uniform per-slot key-chunk counts (all cores)
WAVE = 512         # sequence positions per DMA wave

_CACHE = {}
LAST_RESULT = None


def _build():
    nc = bacc.Bacc(
        "TRN2",
        target_bir_lowering=False,
        debug=False,
        enable_asserts=True,
        num_devices=8,
    )

    xqt_d = nc.declare_dram_parameter("xqt", [D, NQ], BF, isOutput=False)
    xkt_d = nc.declare_dram_parameter("xkt", [D, S], BF, isOutput=False)
    xvt_d = nc.declare_dram_parameter("xvt", [D, S], BF, isOutput=False)
    wq = nc.declare_dram_parameter("wq", [D, E], BF, isOutput=False)  # pre-scaled 1/8
    wk = nc.declare_dram_parameter("wk", [D, E], BF, isOutput=False)
    wv = nc.declare_dram_parameter("wv", [D, E], BF, isOutput=False)
    masks = nc.declare_dram_parameter("masks", [P, 8 * QBLK], BF, isOutput=False)
    zout = nc.declare_dram_parameter("z", [E, NQ], F32, isOutput=True)

    with tile.TileContext(nc) as tc:
        with (
            tc.tile_pool(name="const", bufs=1) as const,
            tc.tile_pool(name="xw", bufs=3) as xw,
            tc.tile_pool(name="proj", bufs=1) as proj,
            tc.tile_pool(name="work", bufs=3) as work,
            tc.tile_pool(name="epi", bufs=2) as epi,
            tc.tile_pool(name="psS", bufs=2, space="PSUM") as psS,
            tc.tile_pool(name="psZ", bufs=2, space="PSUM") as psZ,
            tc.tile_pool(name="psM", bufs=2, space="PSUM") as psM,
        ):
            # ---- weights first (small, needed by every projection) ----
            wq_sb = const.tile([P, NCHUNK_D, E], BF, tag="wq")
            wk_sb = const.tile([P, NCHUNK_D, E], BF, tag="wk")
            wv_sb = const.tile([P, NCHUNK_D, E], BF, tag="wv")
            for w_dram, w_sb in ((wk, wk_sb), (wv, wv_sb), (wq, wq_sb)):
                nc.sync.dma_start(
                    out=w_sb[:, :, :],
                    in_=w_dram.rearrange("(c p) e -> p c e", p=P),
                )
            masks_sb = const.tile([P, 8 * QBLK], BF, tag="masks")

            # ---- X^T wave tiles: DMA paced by slot rotation ----
            kwaves, vwaves, qwaves = [], [], []

            def x_wave(src_d, w, waves, tag):
                t = xw.tile([P, NCHUNK_D, WAVE], BF, tag=tag)
                nc.sync.dma_start(
                    out=t[:, :, :],
                    in_=src_d[:, w * WAVE : (w + 1) * WAVE].rearrange(
                        "(c p) r -> p c r", p=P
                    ),
                )
                waves.append(t)

            # priority-ordered issue: attn block 0 deps first (k0,k1,q0,masks),
            # then remaining waves, early seq positions first
            x_wave(xkt_d, 0, kwaves, "kx")
            x_wave(xkt_d, 1, kwaves, "kx")
            x_wave(xqt_d, 0, qwaves, "qx")
            nc.sync.dma_start(out=masks_sb[:, :], in_=masks[:, :])
            x_wave(xvt_d, 0, vwaves, "vx")
            x_wave(xvt_d, 1, vwaves, "vx")
            x_wave(xqt_d, 1, qwaves, "qx")
            for w in range(2, S // WAVE):
                x_wave(xkt_d, w, kwaves, "kx")
                x_wave(xvt_d, w, vwaves, "vx")
                if w < NQ // WAVE:
                    x_wave(xqt_d, w, qwaves, "qx")

            # ---- projections ----
            # kt2 packed: even key chunk j=2m -> rows 0:64 cols [m*128,(m+1)*128)
            #             odd  key chunk j=2m+1 -> rows 64:128 same cols
            kt2 = proj.tile([P, S // 2], BF, tag="ktp")
            qt2 = proj.tile([P, NQ], BF, tag="qt")  # both halves = Q^T
            vp = proj.tile([P, NKCH, E + 1], BF, tag="vp")
            nc.gpsimd.memset(vp[:, :, E : E + 1], 1.0)

            def proj_k_block(g):
                # keys [g*512, (g+1)*512) -> kt2 packed cols [g*256, (g+1)*256)
                ps = psM.tile([E, QBLK], F32, tag="m")
                for c in range(NCHUNK_D):
                    nc.tensor.matmul(
                        ps,
                        lhsT=wk_sb[:, c, :],
                        rhs=kwaves[g][:, c, :],
                        start=(c == 0),
                        stop=(c == NCHUNK_D - 1),
                    )
                psv = ps.rearrange("e (m h x) -> e h m x", h=2, x=P)
                for h in range(2):
                    dst = kt2[
                        h * E : (h + 1) * E,
                        g * (QBLK // 2) : (g + 1) * (QBLK // 2),
                    ].rearrange("e (m x) -> e m x", x=P)
                    nc.vector.tensor_copy(dst, psv[:, h])

            def proj_q_block(g):
                ps = psM.tile([E, QBLK], F32, tag="m")
                for c in range(NCHUNK_D):
                    nc.tensor.matmul(
                        ps,
                        lhsT=wq_sb[:, c, :],
                        rhs=qwaves[g][:, c, :],
                        start=(c == 0),
                        stop=(c == NCHUNK_D - 1),
                    )
                nc.vector.tensor_copy(qt2[0:E, g * QBLK : (g + 1) * QBLK], ps)
                nc.vector.tensor_copy(qt2[E : 2 * E, g * QBLK : (g + 1) * QBLK], ps)

            def proj_v_tile(i):
                ps = psM.tile([P, E], F32, tag="m")
                w, r = i // 4, i % 4
                for c in range(NCHUNK_D):
                    nc.tensor.matmul(
                        ps,
                        lhsT=vwaves[w][:, c, r * P : (r + 1) * P],
                        rhs=wv_sb[:, c, :],
                        start=(c == 0),
                        stop=(c == NCHUNK_D - 1),
                    )
                nc.vector.tensor_copy(vp[:, i, 0:E], ps)

            # emit all projections in wave/consumption order (higher PE
            # priority than attention, so the load pipeline keeps draining)
            for g in range(S // QBLK):
                proj_k_block(g)
                for i in range(4 * g, 4 * g + 4):
                    proj_v_tile(i)
                if g < NQ // QBLK:
                    proj_q_block(g)

            # ---- attention ----
            def attn_block(ib):
                jmax = SLOT_J[ib]
                qloc = ib * QBLK
                zps = psZ.tile([E + 1, QBLK], F32, tag="zt")

                def emit_pv(pt, jp):
                    for h in range(2):
                        j = 2 * jp + h
                        nc.tensor.matmul(
                            zps,
                            lhsT=vp[:, j, :],
                            rhs=pt[:, h * QBLK : (h + 1) * QBLK],
                            start=(j == 0),
                            stop=(j == jmax - 1),
                            skip_group_check=True,
                        )

                prev = None
                for jp in range(jmax // 2):
                    sps = psS.tile([P, 2 * QBLK], F32, tag="st")
                    for h in range(2):
                        nc.tensor.matmul(
                            sps[:, h * QBLK : (h + 1) * QBLK],
                            lhsT=kt2[h * E : (h + 1) * E, jp * P : (jp + 1) * P],
                            rhs=qt2[h * E : (h + 1) * E, qloc : qloc + QBLK],
                            start=True,
                            stop=True,
                            tile_position=(h * E, 0),
                        )
                    pt = work.tile([P, 2 * QBLK], BF, tag="pt")
                    nc.scalar.activation(
                        out=pt, in_=sps, func=mybir.ActivationFunctionType.Exp
                    )
                    j0 = 2 * jp
                    if j0 >= jmax - 8:
                        m = j0 - (jmax - 8)
                        nc.vector.tensor_mul(
                            pt, pt, masks_sb[:, m * QBLK : (m + 2) * QBLK]
                        )
                    if prev is not None:
                        emit_pv(*prev)
                    prev = (pt, jp)
                emit_pv(*prev)

                # normalize in transposed layout; host undoes the transpose
                zsb = epi.tile([E + 1, QBLK], F32, tag="zsb")
                nc.vector.tensor_copy(zsb, zps)
                rc = epi.tile([1, QBLK], F32, tag="rc")
                nc.vector.reciprocal(rc, zsb[E : E + 1, :])
                rb = epi.tile([E, QBLK], F32, tag="rb")
                nc.gpsimd.partition_broadcast(rb, rc)
                zt = epi.tile([E, QBLK], F32, tag="zt")
                nc.vector.tensor_mul(zt, zsb[0:E, :], rb)
                nc.sync.dma_start(out=zout[:, qloc : qloc + QBLK], in_=zt)

            for ib in range(4):
                attn_block(ib)

    nc.compile()
    return nc


def _get_nc():
    if "nc" not in _CACHE:
        _CACHE["nc"] = _build()
    return _CACHE["nc"]


def _ensure_ntff_hook():
    """Install antenv.axon_hooks + NTFF profile hook if the image lacks it."""
    import types

    try:
        from antenv import axon_hooks  # noqa: F401

        return
    except ImportError:
        pass
    import antenv
    from concourse import bass_utils as _bu

    mod = types.ModuleType("antenv.axon_hooks")
    _state = {}
    mod.set_axon_ntff_profile_hook = lambda h: _state.__setitem__("h", h)
    mod.get_axon_ntff_profile_hook = lambda: _state.get("h")
    sys.modules["antenv.axon_hooks"] = mod
    antenv.axon_hooks = mod
    sys.path.insert(0, "/root/.axon_site/trn_agent_boot")
    from trn_boot import _ntff_profile_via_ctypes

    mod.set_axon_ntff_profile_hook(
        _ntff_profile_via_ctypes("/opt/axon/libaxon_pjrt.so")
    )
    _bu.upload_artifacts = lambda tmpdir: f"local://{tmpdir}"


def _make_masks(h):
    kl = np.arange(P)[:, None]
    ql = np.arange(QBLK)[None, :]
    diag = [(kl <= ql - P * t).astype(np.float32) for t in range(4)]
    ones = np.ones((P, QBLK), np.float32)
    zero = np.zeros((P, QBLK), np.float32)
    tiles = diag + [zero] * 4 if h == 0 else [ones] * 4 + diag
    return np.concatenate(tiles, axis=1).astype(BF16)


def kernel(key_inputs, value_inputs, query_inputs, Wq, Wk, Wv):
    global LAST_RESULT
    import os

    key_inputs = np.asarray(key_inputs, dtype=np.float32)
    value_inputs = np.asarray(value_inputs, dtype=np.float32)
    query_inputs = np.asarray(query_inputs, dtype=np.float32)
    wq_b = (np.asarray(Wq, dtype=np.float32) * 0.125).astype(BF16)
    wk_b = np.asarray(Wk, dtype=np.float32).astype(BF16)
    wv_b = np.asarray(Wv, dtype=np.float32).astype(BF16)
    masks_np = [_make_masks(0), _make_masks(1)]

    in_maps = []
    for c in range(8):
        b, h = c // 2, c % 2
        xq_c = np.concatenate(
            [query_inputs[b, q0 : q0 + QBLK] for q0 in QSTARTS[h]], axis=0
        )
        xk_c = key_inputs[b]
        xv_c = value_inputs[b]
        in_maps.append(
            {
                "xqt": np.ascontiguousarray(xq_c.T).astype(BF16),
                "xkt": np.ascontiguousarray(xk_c.T).astype(BF16),
                "xvt": np.ascontiguousarray(xv_c.T).astype(BF16),
                "wq": wq_b,
                "wk": wk_b,
                "wv": wv_b,
                "masks": masks_np[h],
            }
        )

    nc = _get_nc()
    trace = bool(int(os.environ.get("KERNEL_TRACE", "0")))
    if trace:
        _ensure_ntff_hook()
    res = run_bass_kernel_spmd(
        nc,
        in_maps,
        core_ids=list(range(8)),
        trace=trace,
        tmpdir=os.environ.get("KERNEL_TRACE_DIR") or None,
    )
    LAST_RESULT = res

    out = np.empty((B, S, E), dtype=np.float32)
    for c in range(8):
        b, h = c // 2, c % 2
        z = np.asarray(res.results[c]["z"], dtype=np.float32)  # [E, NQ]
        for ib, q0 in enumerate(QSTARTS[h]):
            out[b, q0 : q0 + QBLK] = z[:, ib * QBLK : (ib + 1) * QBLK].T
    return out
